# revision 42
# baseline (speedup 1.0000x reference)
"""Trainium2 Bass kernel for a single pre-norm transformer block.

Reference math (B=4, T=2048, C=512, H=8, D=64, fp32):
    h  = LN(x; g1, b1) ; q,k,v = h @ Wq/Wk/Wv (per head)
    wei = softmax_over_QUERY_axis( causal_mask(q k^T / sqrt(C)) )
    x2  = x + concat_heads(wei @ v) @ Wo + bo
    out = x2 + relu(LN(x2; g2, b2) @ W1 + b1) @ W2 + b2

Sharding over 8 NeuronCores: pairs of cores per batch element
(core = 2*b + r). Each core computes LN1 + QKV for its batch,
attention for its 4 heads (global heads 4r..4r+3), the partial output
projection over its heads' features, a pairwise ReduceScatter (8
chunks of 256 tokens), and the token-parallel FFN on its 1024 tokens.

Precision strategy (validated against the fp32 reference, rel ~6e-3):
  - activations flow bf16; fp32 PSUM accumulation everywhere
  - scores: q,k quantized fp8 (e4m3), DoubleRow over the head dim
    (2 k-tiles of 32) -> 0.5 cycles/row
  - QKV + FFN1 matmuls: "residual fp8" DoubleRow: x ~= x_hi + x_lo
    (both fp8, same scale), W ~= W_hi + W_lo;  x@W ~= x_hi@(W_hi+W_lo)
    + x_lo@W_hi  (3 DR matmuls per 2 chunks = 0.75x bf16 cycles,
    bf16-grade accuracy; only the negligible lo*lo term is dropped)
  - AV: keys < 1024 use fp8 exps (x) residual-fp8 v/Z rows via
    DoubleRow (0.5x); late keys (small Z -> outlier risk) stay bf16
  - FFN2 + output projection: bf16

Schedule: key-block-major attention (exact causal strips [128 keys x
(T-128*ii) queries], softmax over the query axis via ACT exp with
accum row sums), AV accumulated query-quarter-major into [128,512]
PSUM strips so the output projection + collective + FFN pipeline per
256-token chunk overlaps later key blocks' exponentials. Element-wise
work is spread across DVE and GpSimd; exps own the ACT engine.
"""

import math
import sys

sys.path.insert(0, "/opt/trn_rl_repo")

import ml_dtypes
import numpy as np

B, T, C, H, D = 4, 2048, 512, 8, 64
EPS = 1e-5
NCORES = 8
TH = T // 2  # tokens per core in the FFN phase
HPC = H // 2  # heads per core
NT = T // 128  # 16 key tiles per batch
NCH = 8  # collective chunks (256 tokens each)
NEG = -1e30
FF = 4 * C
LN16 = math.log(16.0)

F8NP = ml_dtypes.float8_e4m3
BFNP = ml_dtypes.bfloat16

_CACHE: dict = {}


def _build_program(cfg, sim=False):
    from contextlib import ExitStack

    import concourse.bacc as bacc
    import concourse.bass as bass
    import concourse.tile as tile
    from concourse import mybir
    from concourse.masks import make_identity

    (has_bqkv, has_bo, has_b2, has_b1, eq, ek, ev, e1) = cfg
    f32 = mybir.dt.float32
    f32r = mybir.dt.float32r
    bf16 = mybir.dt.bfloat16
    f8 = mybir.dt.float8e4
    AF = mybir.ActivationFunctionType
    ALU = mybir.AluOpType
    DR = mybir.MatmulPerfMode.DoubleRow

    # psum descale immediates (see _shard_inputs for the scale ledger)
    dq = 256.0 / (16.0 * (2.0 ** eq))
    dk = 1.0 / (2.0 ** ek)
    dv = 4.0 / (2.0 ** ev)

    nc = bacc.Bacc(
        "TRN2", target_bir_lowering=False, debug=False,
        num_devices=1 if sim else NCORES,
    )

    x_e = nc.dram_tensor("x", [T, C], bf16, kind="ExternalInput").ap()
    xh_e = nc.dram_tensor("xh", [TH, C], bf16, kind="ExternalInput").ap()
    wq_e = nc.dram_tensor("wq", [128, 4, 2, 2, 128], f8,
                          kind="ExternalInput").ap()
    wk_e = nc.dram_tensor("wk", [128, 4, 2, 2, 128], f8,
                          kind="ExternalInput").ap()
    wv_e = nc.dram_tensor("wv", [128, 4, 2, 256], f8,
                          kind="ExternalInput").ap()
    wo_e = nc.dram_tensor("wo", [128, 2, C], bf16, kind="ExternalInput").ap()
    w1_e = nc.dram_tensor("w1", [128, 4, 2, FF], f8,
                          kind="ExternalInput").ap()
    w2_e = nc.dram_tensor("w2", [128, 16, C], bf16, kind="ExternalInput").ap()
    if has_b1:
        b1_e = nc.dram_tensor("b1", [FF], f32, kind="ExternalInput").ap()
    if has_bqkv:
        bq_e = nc.dram_tensor("bq", [1, 2, 128], f32, kind="ExternalInput").ap()
        bk_e = nc.dram_tensor("bk", [1, 2, 128], f32, kind="ExternalInput").ap()
        bv_e = nc.dram_tensor("bv", [1, 256], f32, kind="ExternalInput").ap()
    if has_bo:
        bo_e = nc.dram_tensor("bo", [C], f32, kind="ExternalInput").ap()
    if has_b2:
        b2_e = nc.dram_tensor("b2", [C], f32, kind="ExternalInput").ap()
    y_e = nc.dram_tensor("y", [TH, C], f32, kind="ExternalOutput").ap()

    CCH = 2 * T // NCH // 2  # 256 rows in, half out
    cc_in = [nc.dram_tensor(f"cc_in{k}", [CCH, C], bf16) for k in range(NCH)]
    cc_out = [nc.dram_tensor(f"cc_out{k}", [CCH // 2, C], bf16)
              for k in range(NCH)]

    def dup2(sl):
        """[128, N] slice -> [128, 2(dup), N] stride-0 AP."""
        assert len(sl.ap) == 2
        return bass.AP(tensor=sl.tensor, offset=sl.offset,
                       ap=[sl.ap[0], [0, 2], sl.ap[1]])

    def bcast(vec_ap):
        """[C] dram/sbuf row -> [128, C] partition-broadcast AP."""
        b_ = vec_ap[None, :]
        return bass.AP(tensor=b_.tensor, offset=b_.offset,
                       ap=[[0, 128], b_.ap[1]])

    with tile.TileContext(nc) as tc, ExitStack() as ctx:
        # ---- psum pools (8 banks total) ----
        sc_ps = ctx.enter_context(tc.tile_pool(name="scps", bufs=1,
                                               space="PSUM"))  # 2 banks
        av_ps = ctx.enter_context(tc.tile_pool(name="avps", bufs=1,
                                               space="PSUM"))  # 1 bank
        sh_ps = ctx.enter_context(tc.tile_pool(name="shps", bufs=1,
                                               space="PSUM"))  # 1 bank
        tp_ps = av_ps  # tp0/tp1 tags live in the av pool
        fp_ps2 = av_ps  # FFN1 psum shares the "av" tag (time-disjoint)

        consts = ctx.enter_context(tc.tile_pool(name="consts", bufs=1))
        smalls = ctx.enter_context(tc.tile_pool(name="smalls", bufs=2))

        # ---- constants ----
        ident_b = consts.tile([128, 128], bf16)
        make_identity(nc, ident_b)
        mb_b = consts.tile([128, 128], bf16)
        nc.gpsimd.memset(mb_b[:], 0.0)
        nc.gpsimd.affine_select(
            out=mb_b[:], in_=mb_b[:], compare_op=ALU.is_ge, fill=NEG,
            base=0, pattern=[[1, 128]], channel_multiplier=-1,
        )
        ln16_t = consts.tile([128, 1], f32)
        nc.vector.memset(ln16_t[:], LN16)
        if has_b1:
            b1_sb = consts.tile([128, 16], f32)
            nc.sync.dma_start(b1_sb[:], b1_e.rearrange("(n p) -> p n", p=128))
        if has_bqkv:
            onesq = consts.tile([1, 512], f32r)
            nc.vector.memset(onesq[:], 1.0)
            bq_sb = consts.tile([1, 2, 128], f32)
            nc.sync.dma_start(bq_sb[:], bq_e)
            bk_sb = consts.tile([1, 2, 128], f32)
            nc.sync.dma_start(bk_sb[:], bk_e)
            bv_sb = consts.tile([1, 256], f32)
            nc.sync.dma_start(bv_sb[:], bv_e)
        if has_bo:
            bo_sb = consts.tile([128, C], f32)
            nc.sync.dma_start(bo_sb[:], bcast(bo_e))
        
        if has_b2:
            b2row_ = consts.tile([1, C], bf16)
            nc.sync.dma_start(b2row_[:], b2_e[None, :])
            onesb_ = consts.tile([1, 128], bf16)
            nc.vector.memset(onesb_[:], 1.0)

        # ---- persistent weights / activations ----
        wpool = ctx.enter_context(tc.tile_pool(name="wts", bufs=1))
        w1_sb = wpool.tile([128, 4, 2, FF], f8)
        w2_sb = wpool.tile([128, 16, C], bf16)
        wo_sb = wpool.tile([128, 2, C], bf16)

        def emit_wdma():
            nc.sync.dma_start(wo_sb[:], wo_e)
            nc.sync.dma_start(w1_sb[:], w1_e)
            nc.sync.dma_start(w2_sb[:], w2_e)

        qkp = ctx.enter_context(tc.tile_pool(name="qk", bufs=1))
        qT = qkp.tile([128, 2, T], f8)   # [32h+i, j, t]
        kT = qkp.tile([128, 2, T], f8)
        v_sb = qkp.tile([128, NT, HPC * D], bf16)  # [s, tile, 64h+d]

        vsp = ctx.enter_context(tc.tile_pool(name="vsp", bufs=1))
        vs8 = [vsp.tile([128, 8, 2, D], f8, name=f"vs8_{h}")
               for h in range(HPC)]
        vs_bf = [vsp.tile([128, NT, D], bf16, name=f"vsbf_{h}")
                 for h in range(HPC)]
        zp = ctx.enter_context(tc.tile_pool(name="zp", bufs=1))
        z = [zp.tile([128, NT], f32, name=f"z_{h}") for h in range(HPC)]
        zr = [zp.tile([128, NT], f32, name=f"zr_{h}") for h in range(HPC)]

        attnp = ctx.enter_context(tc.tile_pool(name="attnp", bufs=1))
        attn = [attnp.tile([128, T], bf16, name=f"attn{p}") for p in range(2)]

        expp = ctx.enter_context(tc.tile_pool(name="expp", bufs=1))
        et = [[None] * NT for _ in range(HPC)]

        late = {}  # pools/tiles opened after the hT scope closes (ii==3)

        i32 = mybir.dt.int32

        def rstd_of(var_ap, rstd, pool, tag, n):
            """rstd = 16/sqrt(var+eps) on DVE: quake seed + 2 Newton steps
            (x16 folded into the last step's constants). All integer
            intermediates stay positive and below 2^31."""
            v = pool.tile([128, n], f32, tag=f"nv{tag}")
            nc.vector.tensor_scalar(v[:], var_ap, EPS, None, ALU.add)
            y = pool.tile([128, n], f32, tag=f"ny{tag}")
            yi = y[:].bitcast(i32)
            # yi = 0x5f3759df - (bits(v) >> 1), int32-safe, no op mixing
            nc.vector.tensor_scalar(yi, v[:].bitcast(i32), 1, None,
                                    ALU.logical_shift_right)
            nc.vector.tensor_scalar(yi, yi, 0x5f3759df, None, ALU.subtract)
            nc.vector.tensor_scalar(yi, yi, -1, None, ALU.mult)
            t = pool.tile([128, n], f32, tag=f"nt{tag}")
            nc.vector.tensor_tensor(t[:], y[:], y[:], ALU.mult)
            nc.vector.tensor_tensor(t[:], t[:], v[:], ALU.mult)
            nc.vector.tensor_scalar(t[:], t[:], -0.5, 1.5, ALU.mult, ALU.add)
            nc.vector.tensor_tensor(y[:], y[:], t[:], ALU.mult)
            nc.vector.tensor_tensor(t[:], y[:], y[:], ALU.mult)
            nc.vector.tensor_tensor(t[:], t[:], v[:], ALU.mult)
            nc.vector.tensor_scalar(t[:], t[:], -8.0, 24.0, ALU.mult, ALU.add)
            nc.vector.tensor_tensor(rstd, y[:], t[:], ALU.mult)

        def layer_norm(xm, hm, tag, pool=None):
            """hm(bf16) = 16*(xm - mean)*rsqrt(var+eps), per token row."""
            pool = pool or smalls
            stats = pool.tile([128, 6], f32, tag=f"bnst{tag}")
            nc.vector.bn_stats(stats[:], xm)
            mv = pool.tile([128, 2], f32, tag=f"bnag{tag}")
            nc.vector.bn_aggr(mv[:], stats[:])
            rstd = pool.tile([128, 1], f32, tag=f"rstd{tag}")
            rstd_of(mv[:, 1:2], rstd[:], pool, tag, 1)
            nc.vector.tensor_scalar(hm, xm, mv[:, 0:1], rstd[:],
                                    ALU.subtract, ALU.mult)

        # ============== emission helpers ==============

        with ExitStack() as phA:
            xstack = ExitStack()
            hstack = ExitStack()
            hTp = hstack.enter_context(tc.tile_pool(name="hTp", bufs=1))
            hT = hTp.tile([128, 4, 2, T], f8)  # [c128, cc, hi/lo, t]
            wqp = hstack.enter_context(tc.tile_pool(name="wqp", bufs=1))
            wq_sb = wqp.tile([128, 4, 2, 2, 128], f8)
            wk_sb = wqp.tile([128, 4, 2, 2, 128], f8)
            wv_sb = wqp.tile([128, 4, 2, 256], f8)

            def emit_wqdma():
                nc.sync.dma_start(wq_sb[:], wq_e)
                nc.sync.dma_start(wk_sb[:], wk_e)
                nc.sync.dma_start(wv_sb[:], wv_e)
            xpool = xstack.enter_context(tc.tile_pool(name="xp", bufs=1))
            x_sb = xpool.tile([128, NT, C], bf16)
            x_r = x_e.rearrange("(n p) c -> p n c", p=128)

            def emit_xdma(half):
                for xc in range(4 * half, 4 * half + 4):
                    nc.sync.dma_start(
                        x_sb[:, 2 * xc:2 * (xc + 1), :],
                        x_r[:, 2 * xc:2 * (xc + 1), :],
                    )

            def emit_ln1_group(g):
                """LN1 for m-tiles 4g..4g+3 with one batched rstd."""
                mv = smalls.tile([128, 4, 2], f32, tag="mvg")
                rstd = smalls.tile([128, 4], f32, tag="rstdg")
                for mi in range(4):
                    m = 4 * g + mi
                    stats = smalls.tile([128, 6], f32, tag=f"bnst{mi}")
                    nc.vector.bn_stats(stats[:], x_sb[:, m, :])
                    nc.vector.bn_aggr(mv[:, mi, :], stats[:])
                rstd_of(mv[:, :, 1], rstd[:], smalls, "g", 4)
                for mi in range(4):
                    m = 4 * g + mi
                    hm = smalls.tile([128, C], bf16, tag="hm")
                    nc.vector.tensor_scalar(
                        hm[:], x_sb[:, m, :], mv[:, mi, 0:1],
                        rstd[:, mi:mi + 1], ALU.subtract, ALU.mult)
                    tp = tp_tile()
                    for cc_ in range(4):
                        nc.tensor.transpose(
                            tp[:, cc_ * 128:(cc_ + 1) * 128],
                            hm[:, cc_ * 128:(cc_ + 1) * 128],
                            ident_b[:],
                        )
                    mc = slice(m * 128, (m + 1) * 128)
                    nc.vector.tensor_copy(hT[:, :, 0, mc], tp)
                    nc.vector.scalar_tensor_tensor(
                        hT[:, :, 1, mc], hT[:, :, 0, mc], -1.0, tp,
                        ALU.mult, ALU.add,
                    )


            def qk_blk(dst, w4, bias_sb, dscale, blk, copy_eng, js=(0, 1)):
                """one 512-col block of q^T or k^T."""
                tsl = slice(blk * 512, (blk + 1) * 512)
                for j in js:
                    qp = sh_ps.tile([128, 512], f32, tag="sh")
                    for cc_ in range(4):
                        nc.tensor.matmul(
                            qp[:], lhsT=w4[:, cc_, j, :, :],
                            rhs=dup2(hT[:, cc_, 0, tsl]), perf_mode=DR,
                            start=(cc_ == 0), stop=False,
                            skip_group_check=False,
                        )
                    for cc_ in (0, 2):
                        nc.tensor.matmul(
                            qp[:], lhsT=w4[:, cc_:cc_ + 2, j, 0, :],
                            rhs=hT[:, cc_:cc_ + 2, 1, tsl], perf_mode=DR,
                            start=False,
                            stop=(cc_ == 2 and bias_sb is None),
                            skip_group_check=False,
                        )
                    if bias_sb is not None:
                        nc.tensor.matmul(
                            qp[:], lhsT=bias_sb[0:1, j, :],
                            rhs=onesq[0:1, 0:512],
                            start=False, stop=True, skip_group_check=False,
                        )
                    nc.vector.tensor_scalar(dst[:, j, tsl], qp[:], dscale,
                                            None, ALU.mult)

            def emit_q_block(blk):
                qk_blk(qT, wq_sb, bq_sb if has_bqkv else None, dq, blk,
                       None)

            def emit_k_block(blk, js=(0, 1)):
                qk_blk(kT, wk_sb, bk_sb if has_bqkv else None, dk, blk,
                       None, js)

            def emit_v_tile(i):
                vp = sh_ps.tile([128, 512], f32, tag="sh")
                vcopy = (nc.scalar.mul if i < 4 else None)
                for cc_ in range(4):
                    nc.tensor.matmul(
                        vp[:, 0:256],
                        lhsT=dup2(hT[:, cc_, 0, i * 128:(i + 1) * 128]),
                        rhs=wv_sb[:, cc_, :, :], perf_mode=DR,
                        start=(cc_ == 0), stop=False,
                        skip_group_check=False,
                    )
                for cc_ in (0, 2):
                    nc.tensor.matmul(
                        vp[:, 0:256],
                        lhsT=hT[:, cc_:cc_ + 2, 1, i * 128:(i + 1) * 128],
                        rhs=wv_sb[:, cc_:cc_ + 2, 0, :], perf_mode=DR,
                        start=False, stop=(cc_ == 2 and not has_bqkv),
                        skip_group_check=False,
                    )
                if has_bqkv:
                    nc.tensor.matmul(
                        vp[:, 0:256], lhsT=onesq[0:1, 0:128],
                        rhs=bv_sb[0:1, :],
                        start=False, stop=True, skip_group_check=False,
                    )
                if i < 4:
                    nc.scalar.mul(v_sb[:, i, :], vp[:, 0:256], dv)
                else:
                    nc.vector.tensor_scalar(v_sb[:, i, :], vp[:, 0:256],
                                            dv, None, ALU.mult)

            # ---------- attention strip (scores + exp) ----------
            scpar = [0]
            tppar = [0]

            def tp_tile():
                tp = tp_ps.tile([128, 512], bf16, tag=f"tp{tppar[0]}",
                                name="tpreg")
                tppar[0] ^= 1
                return tp[:]

            zpart_state = {}

            def emit_strip(h, ii, which="AB"):
                c0 = 128 * ii
                W = T - c0
                is8 = ii < 8
                if "A" in which:
                    etile = expp.tile([128, W], f8 if is8 else bf16,
                                      name=f"et_{h}_{ii}")
                    et[h][ii] = etile
                else:
                    etile = et[h][ii]
                if c0 < 1024:
                    pieces = [("A", 0, 1024 - c0), ("B", 1024 - c0,
                                                    W - (1024 - c0))]
                else:
                    pieces = [("A", 0, W)]
                pieces = [p for p in pieces if p[0] in which]
                for nm, o, w in pieces:
                    ps = sc_ps.tile([128, 1024], f32,
                                    tag=f"sc{scpar[0]}")
                    scpar[0] ^= 1
                    nchunks = (w + 511) // 512
                    for ch in range(nchunks):
                        co = o + ch * 512
                        cw = min(512, o + w - co)
                        first = ch == 0
                        nc.tensor.matmul(
                            ps[:, ch * 512:ch * 512 + cw],
                            lhsT=kT[32 * h:32 * h + 32, :, c0:c0 + 128],
                            rhs=qT[32 * h:32 * h + 32, :,
                                   c0 + co:c0 + co + cw],
                            perf_mode=DR, tile_position=(32 * h, 0),
                            start=True, stop=not (first and nm == "A"),
                            skip_group_check=False,
                        )
                        if first and nm == "A":
                            nc.tensor.matmul(
                                ps[:, 0:128], lhsT=ident_b[:], rhs=mb_b[:],
                                start=False, stop=True, skip_group_check=False,
                            )
                    if len(pieces) == 1 and nm == "A" and c0 >= 1024:
                        acc = z[h][:, ii:ii + 1]
                    else:
                        acc = smalls.tile([128, 1], f32, tag=f"zpt{nm}{h}")
                        zpart_state[(h, ii, nm)] = acc
                    nc.scalar.activation(
                        etile[:, o:o + w], ps[:, 0:w], AF.Exp,
                        bias=ln16_t[:], scale=1.0 / 4096.0, accum_out=acc[:],
                    )
                if "B" in which and c0 < 1024:
                    nc.vector.tensor_tensor(
                        z[h][:, ii:ii + 1], zpart_state[(h, ii, "A")][:],
                        zpart_state[(h, ii, "B")][:], ALU.add)

            # ------- chunk sub-boundary: vs, AV half, attn, proj, cc -------
            def emit_sub_vs(c):
                for h in range(HPC):
                    nc.vector.reciprocal(zr[h][:, 2 * c:2 * c + 2],
                                         z[h][:, 2 * c:2 * c + 2])
                for h in range(HPC):
                    for i in (2 * c, 2 * c + 1):
                        vsl = v_sb[:, i, 64 * h:64 * h + 64]
                        zc = zr[h][:, i:i + 1]
                        if i < 8 and h % 2 == 0:
                            nc.vector.tensor_scalar(
                                vs8[h][:, i, 0, :], vsl, zc, None, ALU.mult)
                            nc.vector.scalar_tensor_tensor(
                                vs8[h][:, i, 1, :], vsl, zc,
                                vs8[h][:, i, 0, :], ALU.mult, ALU.subtract)
                        else:
                            nc.vector.tensor_scalar(
                                vs_bf[h][:, i, :], vsl, zc, None,
                                ALU.mult)

            def emit_sub_av(c, p):
                g0 = 256 * c
                last_ii = 2 * c + 1
                av = av_ps.tile([128, 256], f32, tag="av",
                                name=f"av_{c}_{p}")
                for u in range(2):
                    for ii2 in range(2 * c + 2):
                        h = 2 * p + u
                        gs = max(g0, 128 * ii2)
                        o = gs - 128 * ii2
                        w = g0 + 256 - gs
                        ao = gs - g0
                        osl = slice(64 * u, 64 * u + 64)
                        start = ii2 == 0
                        stop = ii2 == last_ii
                        if ii2 < 8 and u == 0:
                            nc.tensor.matmul(
                                av[osl, ao:ao + w],
                                lhsT=vs8[h][:, ii2, :, :],
                                rhs=dup2(et[h][ii2][:, o:o + w]),
                                perf_mode=DR, start=start, stop=stop,
                                skip_group_check=False,
                            )
                        else:
                            nc.tensor.matmul(
                                av[osl, ao:ao + w],
                                lhsT=vs_bf[h][:, ii2, :],
                                rhs=et[h][ii2][:, o:o + w],
                                start=start, stop=stop,
                                skip_group_check=False,
                            )
                nc.vector.tensor_scalar(
                    attn[p][:, g0:g0 + 256], av[:], 1.0 / 64.0, None,
                    ALU.mult)

            def emit_sub_proj(c):
                for mi, mm in enumerate((2 * c, 2 * c + 1)):
                    pp = sh_ps.tile([128, 512], f32, tag="sh")
                    msl = slice(mm * 128, (mm + 1) * 128)
                    nc.tensor.matmul(pp[:], lhsT=attn[0][:, msl],
                                     rhs=wo_sb[:, 0, :],
                                     start=True, stop=False)
                    nc.tensor.matmul(pp[:], lhsT=attn[1][:, msl],
                                     rhs=wo_sb[:, 1, :],
                                     start=False, stop=True)
                    pj = smalls.tile([128, 512], bf16, tag="pj")
                    nc.vector.tensor_copy(pj[:], pp[:])
                    nc.sync.dma_start(
                        cc_in[c].ap()[mi * 128:(mi + 1) * 128, :], pj[:])
                if sim:
                    nc.sync.dma_start(cc_out[c].ap(),
                                      cc_in[c].ap()[:CCH // 2, :])
                else:
                    nc.gpsimd.collective_compute(
                        "ReduceScatter", ALU.add,
                        replica_groups=[[0, 1], [2, 3], [4, 5], [6, 7]],
                        ins=[cc_in[c].ap()],
                        outs=[cc_out[c].ap()],
                    )

            # ---------- FFN chunk closures ----------
            def ffn_chunk_pre(k):
                fsm, x2 = late["fsmalls"], late["x2"]
                h2T = late["h2p"].tile([128, 4, 2, 128], f8, tag="h2T")
                relu = late["relup"].tile([128, 16, 128], bf16, tag="relu")
                pt = fsm.tile([128, 512], bf16, tag="pr")
                nc.sync.dma_start(pt[:], cc_out[k].ap())
                xhc = fsm.tile([128, 512], bf16, tag="xh")
                nc.scalar.dma_start(
                    xhc[:],
                    xh_e.rearrange("(n p) c -> p n c", p=128)[:, k, :])
                nc.vector.tensor_tensor(x2[:, k, :], xhc[:], pt[:], ALU.add)
                if has_bo:
                    nc.vector.tensor_tensor(x2[:, k, :], x2[:, k, :],
                                            bo_sb[:], ALU.add)
                hm2 = fsm.tile([128, 512], bf16, tag="hm2")
                layer_norm(x2[:, k, :], hm2[:], "2", pool=fsm)
                tp = tp_tile()
                for cc_ in range(4):
                    nc.tensor.transpose(
                        tp[:, cc_ * 128:(cc_ + 1) * 128],
                        hm2[:, cc_ * 128:(cc_ + 1) * 128],
                        ident_b[:],
                    )
                nc.vector.tensor_copy(h2T[:, :, 0, :], tp)
                nc.vector.scalar_tensor_tensor(
                    h2T[:, :, 1, :], h2T[:, :, 0, :], -1.0, tp,
                    ALU.mult, ALU.add)
                return h2T, relu

            fppar = [0]

            def ffn1_g(state, g):
                """4 ff-tiles (nn = 4g..4g+3) of one chunk."""
                h2T, relu = state
                pool = fp_ps2 if fppar[0] else sh_ps
                fp = pool.tile([128, 4, 128], f32,
                               tag="av" if fppar[0] else "sh", name="fp")
                fppar[0] ^= 1
                for f in range(4):
                    nn = 4 * g + f
                    fsl = slice(nn * 128, (nn + 1) * 128)
                    for cc_ in range(4):
                        nc.tensor.matmul(
                            fp[:, f, :], lhsT=w1_sb[:, cc_, :, fsl],
                            rhs=dup2(h2T[:, cc_, 0, :]), perf_mode=DR,
                            start=(cc_ == 0), stop=False,
                            skip_group_check=False,
                        )
                    for cc_ in (0, 2):
                        nc.tensor.matmul(
                            fp[:, f, :], lhsT=w1_sb[:, cc_:cc_ + 2, 0, fsl],
                            rhs=h2T[:, cc_:cc_ + 2, 1, :], perf_mode=DR,
                            start=False, stop=(cc_ == 2),
                            skip_group_check=False,
                        )
                if has_b1:
                    for f in range(4):
                        nn = 4 * g + f
                        nc.vector.tensor_scalar(
                            relu[:, nn, :], fp[:, f, :], b1_sb[:, nn:nn + 1],
                            0.0, ALU.add, ALU.max)
                else:
                    nc.vector.tensor_scalar(
                        relu[:, 4 * g:4 * g + 4, :], fp[:], 0.0, None,
                        ALU.max)

            def ffn2_fin(state, k):
                h2T, relu = state
                f2 = sh_ps.tile([128, 512], f32, tag="sh")
                for nn in range(16):
                    nc.tensor.matmul(
                        f2[:], lhsT=relu[:, nn, :], rhs=w2_sb[:, nn, :],
                        start=(nn == 0), stop=False,
                        skip_group_check=False,
                    )
                nc.tensor.matmul(
                    f2[:], lhsT=ident_b[:], rhs=late["x2"][:, k, :],
                    start=False, stop=not has_b2, skip_group_check=False,
                )
                if has_b2:
                    nc.tensor.matmul(
                        f2[:], lhsT=onesb_[0:1, :], rhs=b2row_[0:1, :],
                        start=False, stop=True, skip_group_check=False,
                    )
                yt = late["fsmalls"].tile([128, 512], f32, tag="yt")
                nc.vector.tensor_copy(yt[:], f2[:])
                nc.scalar.dma_start(y_e[k * 128:(k + 1) * 128, :], yt[:])

            # ============== the schedule ==============
            fillers = []

            def push_ffn_chunk(k):
                state = []
                fillers.insert(
                    min(4, len(fillers)),
                    lambda: state.append(ffn_chunk_pre(k)))
                cs = []
                for g_ in range(4):
                    cs.append(lambda g=g_: ffn1_g(state[0], g))
                cs.append(lambda: ffn2_fin(state[0], k))
                fillers.extend(cs)

            def pop_filler():
                if fillers:
                    fillers.pop(0)()

            emit_xdma(0)
            emit_wqdma()
            emit_xdma(1)
            emit_ln1_group(0)
            emit_q_block(0)
            emit_k_block(0)
            emit_ln1_group(1)
            emit_q_block(1)
            for h in range(HPC):
                emit_strip(h, 0, "A")
            emit_ln1_group(2)
            emit_q_block(2)
            emit_ln1_group(3)
            emit_q_block(3)
            xstack.close()
            emit_wdma()
            for h in range(HPC):
                emit_strip(h, 0, "B")
                if h < 2:
                    emit_v_tile(h)
            for i in range(2, 4):
                emit_v_tile(i)
            pre = []
            for blk in (1, 2, 3):
                for j in (0, 1):
                    pre.append(lambda b=blk, j=j: emit_k_block(b, (j,)))
            pre += [lambda i=i: emit_v_tile(i) for i in range(4, 16)]
            fillers.extend(pre)

            for ii in range(1, NT):
                for h in range(HPC):
                    emit_strip(h, ii)
                    pop_filler()
                    if ii >= 8:
                        pop_filler()
                    if ii >= 11:
                        pop_filler()
                    if ii >= 13:
                        pop_filler()
                if ii == 3:
                    while fillers:
                        pop_filler()
                    hstack.close()
                    late["fsmalls"] = phA.enter_context(
                        tc.tile_pool(name="fsmalls", bufs=2))
                    x2p = phA.enter_context(tc.tile_pool(name="x2p", bufs=1))
                    late["x2"] = x2p.tile([128, 8, C], bf16, name="x2")
                    late["h2p"] = phA.enter_context(
                        tc.tile_pool(name="h2p", bufs=2))
                    late["relup"] = phA.enter_context(
                        tc.tile_pool(name="relup", bufs=2))
                if ii % 2 == 1:
                    c = ii // 2
                    fillers.insert(0, lambda c=c: emit_sub_vs(c))
                    fillers.insert(1, lambda c=c: emit_sub_av(c, 0))
                    fillers.insert(2, lambda c=c: emit_sub_av(c, 1))
                    fillers.insert(3, lambda c=c: emit_sub_proj(c))
                    if ii == 3:
                        push_ffn_chunk(0)
                        push_ffn_chunk(1)
                    elif ii > 3:
                        push_ffn_chunk(c)
            while fillers:
                pop_filler()

    nc.compile()
    return nc


def _make_runner(nc):
    """Build a cached jitted SPMD callable (adapted from
    bass2jax.run_bass_via_pjrt, so repeat timing calls skip re-tracing)."""
    import jax
    import numpy as np
    from jax.experimental.shard_map import shard_map
    from jax.sharding import Mesh, PartitionSpec

    from concourse import bass2jax, mybir

    bass2jax.install_neuronx_cc_hook()
    assert nc.dbg_addr is None
    partition_name = (
        nc.partition_id_tensor.name if nc.partition_id_tensor else None
    )

    in_names, out_names, out_avals, zero_shapes = [], [], [], []
    for alloc in nc.m.functions[0].allocations:
        if not isinstance(alloc, mybir.MemoryLocationSet):
            continue
        name = alloc.memorylocations[0].name
        if alloc.kind == "ExternalInput":
            if name != partition_name:
                in_names.append(name)
        elif alloc.kind == "ExternalOutput":
            out_names.append(name)
            shape = tuple(alloc.tensor_shape)
            dtype = mybir.dt.np(alloc.dtype)
            out_avals.append(jax.core.ShapedArray(shape, dtype))
            zero_shapes.append((shape, dtype))
    n_params = len(in_names)
    n_outs = len(out_avals)
    all_names = in_names + out_names
    if partition_name is not None:
        all_names = all_names + [partition_name]

    def _body(*args):
        operands = list(args)
        if partition_name is not None:
            operands.append(bass2jax.partition_id_tensor())
        outs = bass2jax._bass_exec_p.bind(
            *operands,
            out_avals=tuple(out_avals),
            in_names=tuple(all_names),
            out_names=tuple(out_names),
            lowering_input_output_aliases=(),
            sim_require_finite=True,
            sim_require_nnan=True,
            nc=nc,
        )
        return tuple(outs)

    devices = jax.devices()[:NCORES]
    mesh = Mesh(np.asarray(devices), ("core",))
    donate = tuple(range(n_params, n_params + n_outs))
    sharded = jax.jit(
        shard_map(
            _body,
            mesh=mesh,
            in_specs=(PartitionSpec("core"),) * (n_params + n_outs),
            out_specs=(PartitionSpec("core"),) * n_outs,
            check_rep=False,
        ),
        donate_argnums=donate,
        keep_unused=True,
    )

    def stage(in_maps):
        concat = [
            np.concatenate(
                [np.ascontiguousarray(m[name]) for m in in_maps], axis=0
            )
            for name in in_names
        ]
        dev_inputs = [jax.device_put(a) for a in concat]
        for a in dev_inputs:
            a.block_until_ready()
        return dev_inputs

    def stage_zeros():
        zeros = [
            jax.device_put(np.zeros((NCORES * s[0],) + tuple(s[1:]), d))
            for (s, d) in zero_shapes
        ]
        for z in zeros:
            z.block_until_ready()
        return zeros

    def execute(dev_inputs, dev_zeros):
        outs = sharded(*dev_inputs, *dev_zeros)
        for o in outs:
            o.block_until_ready()
        return outs

    def run(in_maps, dev_inputs=None):
        """Returns (per_core_outputs, dev_inputs_for_reuse)."""
        if dev_inputs is None:
            dev_inputs = stage(in_maps)
        outs = execute(dev_inputs, stage_zeros())
        outs = [np.asarray(o) for o in outs]
        per_core = []
        for c in range(NCORES):
            d = {}
            for i, name in enumerate(out_names):
                rows = zero_shapes[i][0][0]
                d[name] = outs[i][c * rows:(c + 1) * rows]
            per_core.append(d)
        return per_core, dev_inputs

    def sharded_call(dev_inputs, dev_zeros):
        return sharded(*dev_inputs, *dev_zeros)

    run.stage = stage
    run.stage_zeros = stage_zeros
    run.execute = execute
    run.sharded_call = sharded_call
    return run


def _pow2(x, target=224.0):
    return int(np.floor(np.log2(target / max(float(np.abs(x).max()), 1e-30))))


def _hilo(w, s):
    hi = np.asarray(w * s, F8NP)
    lo = np.asarray(w * s - hi.astype(np.float32), F8NP)
    return hi, lo


def _shard_inputs(inputs):
    x = np.asarray(inputs["x"], np.float32)
    Wq = np.asarray(inputs["Wq"], np.float32)
    Wk = np.asarray(inputs["Wk"], np.float32)
    Wv = np.asarray(inputs["Wv"], np.float32)
    Wo = np.asarray(inputs["Wo"], np.float32)
    bo = np.asarray(inputs["bo"], np.float32)
    W1 = np.asarray(inputs["W1"], np.float32)
    b1 = np.asarray(inputs["b1"], np.float32)
    W2 = np.asarray(inputs["W2"], np.float32)
    b2 = np.asarray(inputs["b2"], np.float32)
    g1 = np.asarray(inputs["g1"], np.float32)
    beta1 = np.asarray(inputs["beta1"], np.float32)
    g2 = np.asarray(inputs["g2"], np.float32)
    beta2 = np.asarray(inputs["beta2"], np.float32)

    scale = C ** -0.5
    # fold LN1 affine into the QKV weights (and the score scale into Wq)
    Wq_f = g1[None, :, None] * Wq * scale  # [H, C, D]
    Wk_f = g1[None, :, None] * Wk
    Wv_f = g1[None, :, None] * Wv
    bq_f = np.einsum("c,hcd->hd", beta1, Wq_f)  # [H, D]
    bk_f = np.einsum("c,hcd->hd", beta1, Wk_f)
    bv_f = np.einsum("c,hcd->hd", beta1, Wv_f)
    W1_f = g2[:, None] * W1
    b1_f = b1 + beta2 @ W1

    eq, ek, ev, e1 = (_pow2(Wq_f), _pow2(Wk_f), _pow2(Wv_f), _pow2(W1_f))
    sq, sk, sv, s1 = 2.0 ** eq, 2.0 ** ek, 2.0 ** ev, 2.0 ** e1

    has_bqkv = bool(
        np.any(bq_f != 0) or np.any(bk_f != 0) or np.any(bv_f != 0)
    )
    has_bo = bool(np.any(bo != 0))
    has_b2 = bool(np.any(b2 != 0))
    has_b1 = bool(np.any(b1_f != 0))
    cfg = (has_bqkv, has_bo, has_b2, has_b1, eq, ek, ev, e1)

    def pack_qk(Wf, hs, s):
        # [H,C,D] head-slice -> [C, 2(j), 128(32h+i)] -> [128, cc, j, hl, 128]
        arr = Wf[hs]                               # [4, C, 64]
        arr = arr.reshape(4, C, 2, 32)             # [h, c, j, i]
        arr = arr.transpose(1, 2, 0, 3).reshape(C, 2, 128)
        hi, lo = _hilo(arr, s)
        out = np.empty((128, 4, 2, 2, 128), F8NP)
        hi = hi.reshape(4, 128, 2, 128)
        lo = lo.reshape(4, 128, 2, 128)
        for cc_ in range(4):
            out[:, cc_, :, 0, :] = hi[cc_]
            out[:, cc_, :, 1, :] = lo[cc_]
        return out

    def pack_v(Wf, hs, s):
        # [H,C,D] -> [C, 256(64h+d)] -> [128, cc, hilo, 256]
        arr = Wf[hs].transpose(1, 0, 2).reshape(C, 256)
        hi, lo = _hilo(arr, s)
        out = np.empty((128, 4, 2, 256), F8NP)
        hi = hi.reshape(4, 128, 256)
        lo = lo.reshape(4, 128, 256)
        for cc_ in range(4):
            out[:, cc_, 0, :] = hi[cc_]
            out[:, cc_, 1, :] = lo[cc_]
        return out

    # W1: [C, FF] -> [128, cc, hilo, FF]
    h1, l1 = _hilo(W1_f, s1)
    w1p = np.empty((128, 4, 2, FF), F8NP)
    h1 = h1.reshape(4, 128, FF)
    l1 = l1.reshape(4, 128, FF)
    for cc_ in range(4):
        w1p[:, cc_, 0, :] = h1[cc_]
        w1p[:, cc_, 1, :] = l1[cc_]

    # W2 folded with the relu scale: [FF, C] -> [128, nn, C]
    w2p = np.ascontiguousarray(
        (W2 / (16.0 * s1)).reshape(16, 128, C).transpose(1, 0, 2)
    ).astype(BFNP)

    in_maps = []
    for c in range(NCORES):
        b, r = c // 2, c % 2
        hs = slice(HPC * r, HPC * (r + 1))
        m = {
            "x": np.ascontiguousarray(x[b]).astype(BFNP),
            "xh": np.ascontiguousarray(np.concatenate([
                x[b, 256 * k + 128 * r:256 * k + 128 * r + 128]
                for k in range(8)
            ])).astype(BFNP),
            "wq": pack_qk(Wq_f, hs, sq),
            "wk": pack_qk(Wk_f, hs, sk),
            "wv": pack_v(Wv_f, hs, sv),
            # Wo rows pair-chunked: [hd-in-chunk, pair, c]
            "wo": np.ascontiguousarray(
                Wo[HPC * D * r:HPC * D * (r + 1)]
                .reshape(2, 128, C).transpose(1, 0, 2)
            ).astype(BFNP),
            "w1": w1p,
            "w2": w2p,
        }
        if has_b1:
            m["b1"] = (16.0 * s1) * b1_f
        if has_bqkv:
            # psum-scale biases: q psum = 16*sq*q_true etc.
            bqa = bq_f[hs].reshape(4, 2, 32).transpose(1, 0, 2).reshape(1, 2, 128)
            bka = bk_f[hs].reshape(4, 2, 32).transpose(1, 0, 2).reshape(1, 2, 128)
            m["bq"] = np.ascontiguousarray(16.0 * sq * bqa)
            m["bk"] = np.ascontiguousarray(16.0 * sk * bka)
            m["bv"] = np.ascontiguousarray(
                (16.0 * sv * bv_f[hs]).reshape(1, 256))
        if has_bo:
            m["bo"] = bo
        if has_b2:
            m["b2"] = b2
        in_maps.append(m)
    return in_maps, cfg


def _get_runner(cfg):
    key = ("runner", cfg)
    if key not in _CACHE:
        nc = _build_program(cfg)
        _CACHE[key] = _make_runner(nc)
    return _CACHE[key]


def kernel(**inputs) -> np.ndarray:
    in_maps, cfg = _shard_inputs(inputs)
    run = _get_runner(cfg)
    per_core, dev_inputs = run(in_maps)
    _CACHE["last"] = (run, in_maps, dev_inputs)
    out = np.empty((B, T, C), np.float32)
    for c in range(NCORES):
        b, r = c // 2, c % 2
        y = per_core[c]["y"]
        for k in range(8):
            lo = 256 * k + 128 * r
            out[b, lo:lo + 128] = y[k * 128:(k + 1) * 128]
    return out


def bench_pipelined(n=10):
    """Dispatch n executions back-to-back (async), return avg seconds/call
    for the last n-1 (first call absorbs queueing)."""
    import time

    run, in_maps, dev_inputs = _CACHE["last"]
    zsets = [run.stage_zeros() for _ in range(n)]
    # warm
    run.execute(dev_inputs, zsets[0])
    t0 = time.perf_counter()
    outs = []
    for i in range(1, n):
        outs.append(run.sharded_call(dev_inputs, zsets[i]))
    for os_ in outs:
        for o in os_:
            o.block_until_ready()
    t1 = time.perf_counter()
    return (t1 - t0) / (n - 1)


def timed_rerun():
    """Re-run the last kernel() invocation with device-resident inputs
    and pre-staged output buffers; returns wall seconds of execute only."""
    import time

    run, in_maps, dev_inputs = _CACHE["last"]
    dev_zeros = run.stage_zeros()
    t0 = time.perf_counter()
    run.execute(dev_inputs, dev_zeros)
    return time.perf_counter() - t0
